# revision 25
# baseline (speedup 1.0000x reference)
"""Trainium2 Bass kernel for nn_EncoderOnlyBlock (4-head full-dim encoder block).

Sharding: data-parallel, 8 cores = (batch b, seq-half), with a pairwise
K^T AllGather so each core projects only its own 1024 tokens through Wk.

v5 = v4 plus:
  - head 0 also uses the K-exchange (no duplicated full-local K^T), so xtf
    is gone entirely; a dummy 128B AllGather issued first absorbs the
    one-time CC bootstrap barrier while x^T loads.
  - z-chain (y @ W2) runs in fp8 DoubleRow (W2 packed as K-pair tiles on
    host), halving its PE time; bias row matmuls unchanged.
  - DMA queue isolation: sync carries only the just-in-time Wq/Wk/Wv tile
    loads; xn/w1/w2 bulk loads moved to vector+gpsimd and prefetched in the
    prologue; xr prefetch depth 4; yt PSUM->SBUF cast on scalar.

See kernel_v2/v3 docstrings for the math derivation.
"""

import numpy as np
import ml_dtypes

BF = ml_dtypes.bfloat16
F8 = ml_dtypes.float8_e4m3
P = 128
D = 1024
S = 2048
SI = 1024
H = 4
ET = D // P       # 8 e/d/f 128-blocks
SJT = S // P      # 16 sj 128-blocks
SIT = SI // P     # 8 si 128-blocks
SCALE = 1.0 / 32.0  # 1/sqrt(D)
LN4 = float(np.log(4.0))
EPS = 1e-5

_CACHE = {}


def _emit(nc, tc, A, trivial_gbe):
    """Emit the per-core program. A: dict name -> dram AP."""
    from contextlib import ExitStack

    import concourse.bass as bass
    import concourse.mybir as mybir
    from concourse.masks import make_identity

    f32 = mybir.dt.float32
    bf16 = mybir.dt.bfloat16
    fp8 = mybir.dt.float8e4
    Act = mybir.ActivationFunctionType
    Alu = mybir.AluOpType
    DR = mybir.MatmulPerfMode.DoubleRow
    PAIRS = [[0, 1], [2, 3], [4, 5], [6, 7]]

    with ExitStack() as ctx:
        consts = ctx.enter_context(tc.tile_pool(name="consts", bufs=1))
        psA = ctx.enter_context(tc.tile_pool(name="psA", bufs=4, space="PSUM"))
        dram = ctx.enter_context(tc.tile_pool(name="dram", bufs=4, space="DRAM"))

        bqr_sb = consts.tile([P, H * ET], f32, tag="bqr")
        nc.sync.dma_start(out=bqr_sb[:], in_=A["bqr"][:])
        buv_sb = consts.tile([1, D], bf16, tag="buv")
        nc.sync.dma_start(out=buv_sb[:], in_=A["buv"][:])
        ones8_sb = consts.tile([P, 2, 16], fp8, tag="ones8")
        nc.sync.dma_start(out=ones8_sb[:], in_=A["ones8"][:])
        ones_sb = consts.tile([1, P], bf16, tag="ones")
        nc.vector.memset(ones_sb[:], 1.0)
        eps_sb = consts.tile([P, 1], f32, tag="eps")
        nc.vector.memset(eps_sb[:], EPS)
        nln4_sb = consts.tile([P, 1], f32, tag="nln4")
        nc.vector.memset(nln4_sb[:], -LN4)
        # per-head, per-si 1/colsum scalars and their staging rows
        rec_sb = consts.tile([P, H, SIT], f32, tag="rec")
        csT_sb = consts.tile([P, H, SIT], f32, tag="csT")
        cs_pool = ctx.enter_context(tc.tile_pool(name="cs", bufs=1))

        xpool = ctx.enter_context(tc.tile_pool(name="xp", bufs=1))
        proj_pool = ctx.enter_context(tc.tile_pool(name="pj", bufs=1))
        wqkv_pool = ctx.enter_context(tc.tile_pool(name="wqkv", bufs=6))
        w1_pool = ctx.enter_context(tc.tile_pool(name="w1", bufs=2))
        kt_pool = ctx.enter_context(tc.tile_pool(name="kt", bufs=2))
        kto_pool = ctx.enter_context(tc.tile_pool(name="kto", bufs=2))

        # x^T (own half only; K-own + Q chains consume it)
        xt_sb = xpool.tile([P, ET, SI], fp8, tag="xt")
        for c in range(ET):
            eng = (nc.gpsimd, nc.scalar)[c % 2]
            eng.dma_start(out=xt_sb[:, c, :], in_=A["xt"][c * P:(c + 1) * P, :])
        # x natural order tile (loads emitted after the prologue weight loads
        # so xt + wk0 win the early HBM bandwidth race; M needs it ~100us in)
        xn_sb = xpool.tile([P, SJT, D], fp8, tag="xn")

        # dummy collective: absorbs the one-time CC bootstrap barrier while
        # x^T loads (emitted after the dma_starts so they issue first)
        warm_i = dram.tile([1, P], fp8, tag="dram", name="warm_i")
        warm_o = dram.tile([2, P], fp8, tag="dram", name="warm_o")
        nc.gpsimd.collective_compute(
            "AllGather", mybir.AluOpType.bypass,
            replica_groups=PAIRS,
            ins=[warm_i.opt()], outs=[warm_o.opt()],
        )

        proj_sb = proj_pool.tile([P, SIT, D], bf16, tag="proj")

        kt_tiles = [None] * H

        def k_exchange(h):
            """Compute K^T for own tokens, AllGather across the pair, read the
            full natural-order K^T back into kt_tiles[h]."""
            kx = dram.tile([D, SI], fp8, tag="dram", name=f"kx{h}")
            kg = dram.tile([2, D, SI], fp8, tag="dram", name=f"kg{h}")
            for c in range(ET):
                wk_c = wqkv_pool.tile([P, ET, P], fp8, tag="wqkv", name=f"wk{h}_{c}")
                nc.sync.dma_start(out=wk_c[:], in_=A["wkb"][h, c])
                ps = psA.tile([P, 1024], f32, tag="psA")
                for nb in range(2):
                    for kp in range(4):
                        nc.tensor.matmul(
                            ps[:, nb * 512:(nb + 1) * 512],
                            lhsT=wk_c[:, 2 * kp:2 * kp + 2, :],
                            rhs=xt_sb[:, 2 * kp:2 * kp + 2, nb * 512:(nb + 1) * 512],
                            start=(kp == 0), stop=(kp == 3),
                            perf_mode=DR,
                        )
                kto = kto_pool.tile([P, SI], fp8, tag="kto", name=f"kto{h}_{c}")
                nc.scalar.copy(kto[:], ps[:])
                nc.scalar.dma_start(out=kx[c * P:(c + 1) * P, :], in_=kto[:])
            nc.gpsimd.collective_compute(
                "AllGather", mybir.AluOpType.bypass,
                replica_groups=PAIRS,
                ins=[kx.opt()], outs=[kg.opt()],
            )
            kt_sb = kt_pool.tile([P, ET, S], fp8, tag="kt", name=f"kt{h}")
            # 16 contiguous 128KB block reads (kg[g] is [ET*P, SI] row-major,
            # so each c-block [P, SI] is one contiguous chunk)
            for g in range(2):
                for c in range(ET):
                    nc.gpsimd.dma_start(
                        out=kt_sb[:, c, g * SI:(g + 1) * SI],
                        in_=kg[g, c * P:(c + 1) * P, :],
                    )
            kt_tiles[h] = kt_sb

        qt_tiles = [None] * H

        def emit_q(h, hp):
            # ---- Q^T = Wq^T @ x^T + bq : [e, si]
            qt_sb = hp["qt"].tile([P, ET, SI], fp8, tag="qt", name=f"qt{h}")
            qt_tiles[h] = qt_sb
            for c in range(ET):
                wq_c = wqkv_pool.tile([P, ET, P], fp8, tag="wqkv", name=f"wq{h}_{c}")
                nc.sync.dma_start(out=wq_c[:], in_=A["wqb"][h, c])
                ps = psA.tile([P, 1024], f32, tag="psA")
                for nb in range(2):
                    for kp in range(4):
                        nc.tensor.matmul(
                            ps[:, nb * 512:(nb + 1) * 512],
                            lhsT=wq_c[:, 2 * kp:2 * kp + 2, :],
                            rhs=xt_sb[:, 2 * kp:2 * kp + 2, nb * 512:(nb + 1) * 512],
                            start=(kp == 0), stop=(kp == 3),
                            perf_mode=DR,
                        )
                nc.vector.tensor_scalar(
                    qt_sb[:, c, :], ps[:],
                    scalar1=bqr_sb[:, h * ET + c:h * ET + c + 1], scalar2=None,
                    op0=Alu.add,
                )

        def emit_head(h, hp, phase_b, skip_q=False):
            """One attention head. hp: dict of per-head pools.
            phase_b: callback(t) emitted after proj chain t+2 (None for h<3)."""
            if not skip_q:
                emit_q(h, hp)
            qt_sb = qt_tiles[h]
            w1_sb = w1_tiles[h]

            # ---- A^T = exp(S^T/sqrt(D) - ln4) per sj-block : [sj, si]
            kt_sb = kt_tiles[h]
            at_sb = hp["at"].tile([P, SJT, SI], fp8, tag="at")
            for j in range(SJT):
                ps = psA.tile([P, 1024], f32, tag="psA")
                for nb in range(2):
                    for kp in range(4):
                        nc.tensor.matmul(
                            ps[:, nb * 512:(nb + 1) * 512],
                            lhsT=kt_sb[:, 2 * kp:2 * kp + 2, j * P:(j + 1) * P],
                            rhs=qt_sb[:, 2 * kp:2 * kp + 2, nb * 512:(nb + 1) * 512],
                            start=(kp == 0), stop=(kp == 3),
                            perf_mode=DR,
                        )
                nc.scalar.activation(
                    out=at_sb[:, j, :], in_=ps[:], func=Act.Exp,
                    scale=SCALE, bias=nln4_sb[:],
                )

            # ---- colsum(A^T) = softmax rowsums (carrying the same 1/4)
            cs_sb = cs_pool.tile([1, SI], f32, tag="cs", name=f"cs{h}")
            for nb in range(2):
                cs_ps = psA.tile([P, 1024], f32, tag="psA", name=f"cs{h}_{nb}")
                for jp in range(8):
                    nc.tensor.matmul(
                        cs_ps[0:16, 0:512],
                        lhsT=ones8_sb[:],
                        rhs=at_sb[:, 2 * jp:2 * jp + 2, nb * 512:(nb + 1) * 512],
                        start=(jp == 0), stop=(jp == 7),
                        perf_mode=DR,
                    )
                nc.vector.tensor_copy(cs_sb[:, nb * 512:(nb + 1) * 512],
                                      cs_ps[0:1, 0:512])
            # [1, SI] row -> [P, SIT] partition layout via a DRAM round-trip on
            # one FIFO DMA queue (write row, gather back transposed), then 1/x
            nc.sync.dma_start(out=A["csr"][h:h + 1, :], in_=cs_sb[:])
            csr_t = bass.AP(
                tensor=A["csr"].tensor, offset=A["csr"].offset + h * SI,
                ap=[[1, P], [P, SIT]],
            )
            nc.sync.dma_start(out=csT_sb[:, h, :], in_=csr_t)
            nc.vector.reciprocal(rec_sb[:, h, :], csT_sb[:, h, :])

            # ---- K-own + exchange (first three are in the prologue)
            if h == 1:
                k_exchange(3)
            if h < H - 1:
                w1n = w1_pool.tile([P, ET, D], fp8, tag="w1", name=f"w1_{h+1}")
                nc.gpsimd.dma_start(out=w1n[:], in_=A["w1"][h + 1])
                w1_tiles[h + 1] = w1n

            # ---- M = x^T @ A^T : [d, si]
            m_sb = hp["m"].tile([P, ET, SI], fp8, tag="m")
            for dc in range(ET):
                ps = psA.tile([P, 1024], f32, tag="psA")
                for nb in range(2):
                    for jp in range(8):
                        nc.tensor.matmul(
                            ps[:, nb * 512:(nb + 1) * 512],
                            lhsT=xn_sb[:, 2 * jp:2 * jp + 2, dc * P:(dc + 1) * P],
                            rhs=at_sb[:, 2 * jp:2 * jp + 2, nb * 512:(nb + 1) * 512],
                            start=(jp == 0), stop=(jp == 7),
                            perf_mode=DR,
                        )
                if h < 3:
                    nc.vector.tensor_copy(m_sb[:, dc, :], ps[:])
                else:
                    nc.scalar.copy(m_sb[:, dc, :], ps[:])

            # ---- head^T = Wv^T @ M : [e, si]
            ht_sb = hp["ht"].tile([P, ET, SI], fp8, tag="ht")
            for eb in range(ET):
                wv_eb = wqkv_pool.tile([P, ET, P], fp8, tag="wqkv", name=f"wv{h}_{eb}")
                nc.sync.dma_start(out=wv_eb[:], in_=A["wvb"][h, eb])
                ps = psA.tile([P, 1024], f32, tag="psA")
                for nb in range(2):
                    for kp in range(4):
                        nc.tensor.matmul(
                            ps[:, nb * 512:(nb + 1) * 512],
                            lhsT=wv_eb[:, 2 * kp:2 * kp + 2, :],
                            rhs=m_sb[:, 2 * kp:2 * kp + 2, nb * 512:(nb + 1) * 512],
                            start=(kp == 0), stop=(kp == 3),
                            perf_mode=DR,
                        )
                nc.scalar.copy(ht_sb[:, eb, :], ps[:])

            # ---- proj += r_h * (head_h @ W1_h)
            for t in range(SIT):
                ps = psA.tile([P, 1024], f32, tag="psA")
                for nb in range(2):
                    for ep in range(4):
                        nc.tensor.matmul(
                            ps[:, nb * 512:(nb + 1) * 512],
                            lhsT=ht_sb[:, 2 * ep:2 * ep + 2, t * P:(t + 1) * P],
                            rhs=w1_sb[:, 2 * ep:2 * ep + 2, nb * 512:(nb + 1) * 512],
                            start=(ep == 0), stop=(ep == 3),
                            perf_mode=DR,
                        )
                if h == 0:
                    nc.vector.tensor_scalar_mul(
                        proj_sb[:, t, :], ps[:], rec_sb[:, 0, t:t + 1],
                    )
                else:
                    nc.vector.scalar_tensor_tensor(
                        out=proj_sb[:, t, :], in0=ps[:],
                        scalar=rec_sb[:, h, t:t + 1],
                        in1=proj_sb[:, t, :], op0=Alu.mult, op1=Alu.add,
                    )
                if phase_b is not None and t >= 2:
                    phase_b(t - 2)
            if phase_b is not None:
                phase_b(SIT - 2)
                phase_b(SIT - 1)
                phase_b(SIT)
                phase_b(SIT + 1)

        w1_tiles = [None] * H

        # -------- prologue: K-exchange for heads 0..2, Q for head 0 --------
        head_ctx = ExitStack()
        hp = {n: head_ctx.enter_context(tc.tile_pool(name=n, bufs=1))
              for n in ("qt", "at", "m", "ht")}
        k_exchange(0)
        k_exchange(1)
        emit_q(0, hp)
        k_exchange(2)
        # xn bulk load now that the hot-path weights are queued
        for j in range(SJT):
            eng = (nc.sync, nc.scalar)[j % 2]
            eng.dma_start(out=xn_sb[:, j, :], in_=A["xn"][j * P:(j + 1) * P, :])
        w1_0 = w1_pool.tile([P, ET, D], fp8, tag="w1", name="w1_0")
        nc.gpsimd.dma_start(out=w1_0[:], in_=A["w1"][0])
        w1_tiles[0] = w1_0

        # ---------------- heads 0..2 ----------------
        for h in range(H - 1):
            emit_head(h, hp, None, skip_q=(h < 1))
        head_ctx.close()

        # w2 row-block tiles for the z-chain (allocated from the space heads
        # 0-2 just freed; DMAs overlap head 3's attention)
        w2_pool = ctx.enter_context(tc.tile_pool(name="w2", bufs=8))
        w2_tiles = []
        for kc in range(ET):
            w2_kc = w2_pool.tile([P, D], bf16, tag="w2", name=f"w2_{kc}")
            nc.gpsimd.dma_start(out=w2_kc[:], in_=A["w2"][kc * P:(kc + 1) * P, :])
            w2_tiles.append(w2_kc)

        # ---------------- head 3 + Phase B interleaved ----------------
        with ExitStack() as lctx:
            hp = {n: lctx.enter_context(tc.tile_pool(name=n + "3", bufs=1))
                  for n in ("qt", "at", "m", "ht")}
            lnp = lctx.enter_context(tc.tile_pool(name="lnp", bufs=1))
            xr_pool = lctx.enter_context(tc.tile_pool(name="xr", bufs=3))
            u_pool = lctx.enter_context(tc.tile_pool(name="up", bufs=3))
            sq_pool = lctx.enter_context(tc.tile_pool(name="sq", bufs=1))
            yt_pool = lctx.enter_context(tc.tile_pool(name="yt", bufs=1))
            st_pool = lctx.enter_context(tc.tile_pool(name="st", bufs=8))
            ot_pool = lctx.enter_context(tc.tile_pool(name="ot", bufs=1))

            if not trivial_gbe:
                gbe_sb = lnp.tile([P, 4, D], f32, tag="gbe")
                gbe_bc = bass.AP(
                    tensor=A["gbe"].tensor, offset=A["gbe"].offset,
                    ap=[[0, P], A["gbe"].ap[0], A["gbe"].ap[1]],
                )
                nc.gpsimd.dma_start(out=gbe_sb[:], in_=gbe_bc)
            y_sb = lnp.tile([P, SIT, D], bf16, tag="y")

            xr_tiles = [None] * SIT

            def fetch_xr(t):
                if t < SIT:
                    xr = xr_pool.tile([P, D], f32, tag="xr", name=f"xr{t}")
                    nc.scalar.dma_start(out=xr[:], in_=A["xres"][t * P:(t + 1) * P, :])
                    xr_tiles[t] = xr

            def ln_stats(src, rsum):
                """-> (mu, rstd) [P,1] tiles from src [P,D] + its row-sum.
                sq runs on scalar; everything else stays on DVE to minimize
                cross-engine hops on the critical path."""
                sq = sq_pool.tile([P, D], fp8, tag="sq")
                sumsq = st_pool.tile([P, 1], f32, tag="sumsq")
                nc.scalar.activation(out=sq[:], in_=src, func=Act.Square,
                                     accum_out=sumsq[:])
                mu = st_pool.tile([P, 1], f32, tag="mu")
                nc.vector.tensor_scalar_mul(mu[:], rsum, 1.0 / D)
                # (rsum*mu - sumsq) = -D*var;  std = sqrt(-1/D * that + eps)
                nv = st_pool.tile([P, 1], f32, tag="nv")
                nc.vector.scalar_tensor_tensor(
                    out=nv[:], in0=rsum, scalar=mu[:], in1=sumsq[:],
                    op0=Alu.mult, op1=Alu.subtract,
                )
                rstd = st_pool.tile([P, 1], f32, tag="rstd")
                nc.scalar.activation(out=rstd[:], in_=nv[:], func=Act.Sqrt,
                                     scale=-1.0 / D, bias=eps_sb[:])
                nc.vector.reciprocal(rstd[:], rstd[:])
                return mu, rstd

            b2_state = {}

            def phase_b(t):
                if t < SIT:
                    phase_b1(t)
                if t >= 1 and t - 1 < SIT:
                    phase_b2(t - 1)

            def phase_b1(t):
                # u1 = (x + cvec) + proj, with row-sum accumulated in the same pass
                u1 = u_pool.tile([P, D], f32, tag="u", name=f"u1_{t}")
                rs1 = st_pool.tile([P, 1], f32, tag="rs")
                nc.vector.scalar_tensor_tensor(
                    out=u1[:], in0=xr_tiles[t][:], scalar=1.0,
                    in1=proj_sb[:, t, :], op0=Alu.mult, op1=Alu.add,
                    accum_out=rs1[:],
                )
                fetch_xr(t + 3)
                mu1, rstd1 = ln_stats(u1[:], rs1[:])
                yt_t = y_sb[:, t, :]
                nc.vector.tensor_scalar(
                    yt_t, u1[:], scalar1=mu1[:], scalar2=rstd1[:],
                    op0=Alu.subtract, op1=Alu.mult,
                )
                if not trivial_gbe:
                    nc.gpsimd.tensor_mul(yt_t, yt_t, gbe_sb[:, 0, :])
                    nc.gpsimd.tensor_add(yt_t, yt_t, gbe_sb[:, 1, :])
                # xbar DMA transpose: yt_tile[p, fb, t] = y[t, fb*128+p]
                yt_tile = yt_pool.tile([P, ET, P], bf16, tag="yt")
                nc.sync.dma_start_transpose(out=yt_tile[:], in_=yt_t)
                # z-chain: u2 = y + yhat @ W2' + bu
                ps = psA.tile([P, 1024], f32, tag="psA")
                for nb in range(2):
                    for kc in range(ET):
                        nc.tensor.matmul(
                            ps[:, nb * 512:(nb + 1) * 512],
                            lhsT=yt_tile[:, kc, :],
                            rhs=w2_tiles[kc][:, nb * 512:(nb + 1) * 512],
                            start=(kc == 0), stop=False,
                        )
                    nc.tensor.matmul(
                        ps[:, nb * 512:(nb + 1) * 512],
                        lhsT=ones_sb[:, :],
                        rhs=buv_sb[:, nb * 512:(nb + 1) * 512],
                        start=False, stop=True,
                    )
                b2_state[t] = ps

            def phase_b2(t):
                ps = b2_state.pop(t)
                u2 = u_pool.tile([P, D], f32, tag="u", name=f"u2_{t}")
                rs2 = st_pool.tile([P, 1], f32, tag="rs")
                nc.vector.scalar_tensor_tensor(
                    out=u2[:], in0=y_sb[:, t, :], scalar=1.0,
                    in1=ps[:], op0=Alu.mult, op1=Alu.add,
                    accum_out=rs2[:],
                )
                mu2, rstd2 = ln_stats(u2[:], rs2[:])
                ot = ot_pool.tile([P, D], f32, tag="ot")
                nc.vector.tensor_scalar(
                    ot[:], u2[:], scalar1=mu2[:], scalar2=rstd2[:],
                    op0=Alu.subtract, op1=Alu.mult,
                )
                if not trivial_gbe:
                    nc.gpsimd.tensor_mul(ot[:], ot[:], gbe_sb[:, 2, :])
                    nc.gpsimd.tensor_add(ot[:], ot[:], gbe_sb[:, 3, :])
                nc.sync.dma_start(out=A["out"][t * P:(t + 1) * P, :], in_=ot[:])

            for t in range(3):
                fetch_xr(t)
            emit_head(H - 1, hp, phase_b)


def _build(trivial_gbe):
    import concourse.bass as bass
    import concourse.mybir as mybir
    import concourse.tile as tile
    from concourse import bacc

    f32 = mybir.dt.float32
    bf16 = mybir.dt.bfloat16
    fp8 = mybir.dt.float8e4

    nc = bacc.Bacc("TRN2", target_bir_lowering=False, debug=False, num_devices=8)
    A = {}

    def din(name, shape, dt):
        A[name] = nc.dram_tensor(name, shape, dt, kind="ExternalInput").ap()

    din("xt", [D, SI], fp8)
    din("xn", [S, D], fp8)
    din("xres", [SI, D], f32)
    din("wqb", [H, ET, P, ET, P], fp8)
    din("wkb", [H, ET, P, ET, P], fp8)
    din("wvb", [H, ET, P, ET, P], fp8)
    din("w1", [H, P, ET, D], fp8)
    din("w2", [D, D], bf16)
    din("bqr", [P, H * ET], f32)
    din("buv", [1, D], bf16)
    din("ones8", [P, 2, 16], fp8)
    A["csr"] = nc.dram_tensor("csr", [H, SI], f32, kind="Internal").ap()
    if not trivial_gbe:
        din("gbe", [4, D], f32)
    A["out"] = nc.dram_tensor("out", [SI, D], f32, kind="ExternalOutput").ap()

    with tile.TileContext(nc) as tc:
        _emit(nc, tc, A, trivial_gbe)
    nc.compile()
    return nc


def _get_nc(trivial_gbe=True):
    key = ("nc", trivial_gbe)
    if key not in _CACHE:
        _CACHE[key] = _build(trivial_gbe)
    return _CACHE[key]


def _prep_inputs(inputs):
    x = np.ascontiguousarray(inputs["embedding_matrix"], dtype=np.float32)
    Wq = np.asarray(inputs["Wq"], np.float32)
    bq = np.asarray(inputs["bq"], np.float32)
    Wv = np.asarray(inputs["Wv"], np.float32)
    bv = np.asarray(inputs["bv"], np.float32)
    Wk = np.asarray(inputs["Wk"], np.float32)
    W1 = np.asarray(inputs["W1"], np.float32)
    b1 = np.asarray(inputs["b1"], np.float32)
    W2 = np.asarray(inputs["W2"], np.float32)
    b2 = np.asarray(inputs["b2"], np.float32)
    g1 = np.asarray(inputs["g1"], np.float32)
    be1 = np.asarray(inputs["be1"], np.float32)
    g2 = np.asarray(inputs["g2"], np.float32)
    be2 = np.asarray(inputs["be2"], np.float32)

    trivial = (
        np.array_equal(g1, np.ones(D, np.float32))
        and np.array_equal(g2, np.ones(D, np.float32))
        and np.array_equal(be1, np.zeros(D, np.float32))
        and np.array_equal(be2, np.zeros(D, np.float32))
    )

    def pack_w(W):  # [H, D, D] -> [H, ET(e-blk), P(d-in), ET(kc), P(e-in)] lhsT
        return np.ascontiguousarray(
            W.reshape(H, ET, P, ET, P).transpose(0, 3, 2, 1, 4).astype(F8)
        )

    wqb = pack_w(Wq)
    wkb = pack_w(Wk)
    wvb = pack_w(Wv)
    # [H*D, D] -> [H, P(e-in), ET(e-blk), D(f)]
    w1b = np.ascontiguousarray(
        W1.reshape(H, ET, P, D).transpose(0, 2, 1, 3).astype(F8)
    )
    w2b = np.ascontiguousarray(W2.astype(BF))
    # bq rearranged so bias for (h, e-block c) is column h*ET+c: [P, H*ET]
    bqr = np.ascontiguousarray(bq.reshape(H, ET, P).transpose(2, 0, 1).reshape(P, H * ET))
    cvec = (b1 + sum(bv[h] @ W1[h * D:(h + 1) * D] for h in range(H)))
    buv = np.ascontiguousarray(b2.reshape(1, D).astype(BF))
    ones8 = np.ones((P, 2, 16), F8)

    shared = {
        "wqb": wqb, "wkb": wkb, "wvb": wvb, "w1": w1b, "w2": w2b,
        "bqr": bqr, "buv": buv, "ones8": ones8,
    }
    if not trivial:
        shared["gbe"] = np.ascontiguousarray(np.stack([g1, be1, g2, be2]))
    in_maps = []
    for core in range(8):
        b, half = core // 2, core % 2
        own = x[b, half * SI:(half + 1) * SI]
        m = dict(shared)
        m["xn"] = np.ascontiguousarray(x[b].astype(F8))   # natural order
        m["xt"] = np.ascontiguousarray(own.T.astype(F8))  # own half only
        m["xres"] = np.ascontiguousarray(own + cvec[None, :])
        in_maps.append(m)
    return trivial, in_maps


def kernel(**inputs):
    from concourse.bass_utils import run_bass_kernel_spmd

    trivial, in_maps = _prep_inputs(inputs)
    nc = _get_nc(trivial)
    res = run_bass_kernel_spmd(nc, in_maps, core_ids=list(range(8)))
    out = np.empty((4, S, D), np.float32)
    for core in range(8):
        b, half = core // 2, core % 2
        out[b, half * SI:(half + 1) * SI] = res.results[core]["out"]
    return out


# revision 26
# speedup vs baseline: 1.0372x; 1.0372x over previous
"""Trainium2 Bass kernel for nn_EncoderOnlyBlock (4-head full-dim encoder block).

Sharding: data-parallel, 8 cores = (batch b, seq-half), with a pairwise
K^T AllGather so each core projects only its own 1024 tokens through Wk.

v5 = v4 plus:
  - head 0 also uses the K-exchange (no duplicated full-local K^T), so xtf
    is gone entirely; a dummy 128B AllGather issued first absorbs the
    one-time CC bootstrap barrier while x^T loads.
  - z-chain (y @ W2) runs in fp8 DoubleRow (W2 packed as K-pair tiles on
    host), halving its PE time; bias row matmuls unchanged.
  - DMA queue isolation: sync carries only the just-in-time Wq/Wk/Wv tile
    loads; xn/w1/w2 bulk loads moved to vector+gpsimd and prefetched in the
    prologue; xr prefetch depth 4; yt PSUM->SBUF cast on scalar.

See kernel_v2/v3 docstrings for the math derivation.
"""

import numpy as np
import ml_dtypes

BF = ml_dtypes.bfloat16
F8 = ml_dtypes.float8_e4m3
P = 128
D = 1024
S = 2048
SI = 1024
H = 4
ET = D // P       # 8 e/d/f 128-blocks
SJT = S // P      # 16 sj 128-blocks
SIT = SI // P     # 8 si 128-blocks
SCALE = 1.0 / 32.0  # 1/sqrt(D)
LN4 = float(np.log(4.0))
EPS = 1e-5

_CACHE = {}


def _emit(nc, tc, A, trivial_gbe):
    """Emit the per-core program. A: dict name -> dram AP."""
    from contextlib import ExitStack

    import concourse.bass as bass
    import concourse.mybir as mybir
    from concourse.masks import make_identity

    f32 = mybir.dt.float32
    bf16 = mybir.dt.bfloat16
    fp8 = mybir.dt.float8e4
    Act = mybir.ActivationFunctionType
    Alu = mybir.AluOpType
    DR = mybir.MatmulPerfMode.DoubleRow
    PAIRS = [[0, 1], [2, 3], [4, 5], [6, 7]]

    with ExitStack() as ctx:
        consts = ctx.enter_context(tc.tile_pool(name="consts", bufs=1))
        psA = ctx.enter_context(tc.tile_pool(name="psA", bufs=3, space="PSUM"))
        psB = ctx.enter_context(tc.tile_pool(name="psB", bufs=2, space="PSUM"))
        dram = ctx.enter_context(tc.tile_pool(name="dram", bufs=4, space="DRAM"))

        ident = consts.tile([P, P], bf16, tag="ident")
        make_identity(nc, ident[:])
        bqr_sb = consts.tile([P, H * ET], f32, tag="bqr")
        nc.sync.dma_start(out=bqr_sb[:], in_=A["bqr"][:])
        buv_sb = consts.tile([1, D], bf16, tag="buv")
        nc.sync.dma_start(out=buv_sb[:], in_=A["buv"][:])
        ones8_sb = consts.tile([P, 2, 16], fp8, tag="ones8")
        nc.sync.dma_start(out=ones8_sb[:], in_=A["ones8"][:])
        ones_sb = consts.tile([1, P], bf16, tag="ones")
        nc.vector.memset(ones_sb[:], 1.0)
        eps_sb = consts.tile([P, 1], f32, tag="eps")
        nc.vector.memset(eps_sb[:], EPS)
        nln4_sb = consts.tile([P, 1], f32, tag="nln4")
        nc.vector.memset(nln4_sb[:], -LN4)
        # per-head, per-si 1/colsum scalars and their staging rows
        rec_sb = consts.tile([P, H, SIT], f32, tag="rec")
        csT_sb = consts.tile([P, H, SIT], f32, tag="csT")
        cs_pool = ctx.enter_context(tc.tile_pool(name="cs", bufs=1))

        xpool = ctx.enter_context(tc.tile_pool(name="xp", bufs=1))
        proj_pool = ctx.enter_context(tc.tile_pool(name="pj", bufs=1))
        wqkv_pool = ctx.enter_context(tc.tile_pool(name="wqkv", bufs=6))
        w1_pool = ctx.enter_context(tc.tile_pool(name="w1", bufs=2))
        kt_pool = ctx.enter_context(tc.tile_pool(name="kt", bufs=2))
        kto_pool = ctx.enter_context(tc.tile_pool(name="kto", bufs=2))

        # x^T (own half only; K-own + Q chains consume it)
        xt_sb = xpool.tile([P, ET, SI], fp8, tag="xt")
        for c in range(ET):
            eng = (nc.gpsimd, nc.scalar)[c % 2]
            eng.dma_start(out=xt_sb[:, c, :], in_=A["xt"][c * P:(c + 1) * P, :])
        # x natural order tile (loads emitted after the prologue weight loads
        # so xt + wk0 win the early HBM bandwidth race; M needs it ~100us in)
        xn_sb = xpool.tile([P, SJT, D], fp8, tag="xn")

        # dummy collective: absorbs the one-time CC bootstrap barrier while
        # x^T loads (emitted after the dma_starts so they issue first)
        warm_i = dram.tile([1, P], fp8, tag="dram", name="warm_i")
        warm_o = dram.tile([2, P], fp8, tag="dram", name="warm_o")
        nc.gpsimd.collective_compute(
            "AllGather", mybir.AluOpType.bypass,
            replica_groups=PAIRS,
            ins=[warm_i.opt()], outs=[warm_o.opt()],
        )

        proj_sb = proj_pool.tile([P, SIT, D], bf16, tag="proj")

        kt_tiles = [None] * H

        def k_exchange(h):
            """Compute K^T for own tokens, AllGather across the pair, read the
            full natural-order K^T back into kt_tiles[h]."""
            kx = dram.tile([D, SI], fp8, tag="dram", name=f"kx{h}")
            kg = dram.tile([2, D, SI], fp8, tag="dram", name=f"kg{h}")
            for c in range(ET):
                wk_c = wqkv_pool.tile([P, ET, P], fp8, tag="wqkv", name=f"wk{h}_{c}")
                nc.sync.dma_start(out=wk_c[:], in_=A["wkb"][h, c])
                ps = psA.tile([P, 1024], f32, tag="psA")
                for nb in range(2):
                    for kp in range(4):
                        nc.tensor.matmul(
                            ps[:, nb * 512:(nb + 1) * 512],
                            lhsT=wk_c[:, 2 * kp:2 * kp + 2, :],
                            rhs=xt_sb[:, 2 * kp:2 * kp + 2, nb * 512:(nb + 1) * 512],
                            start=(kp == 0), stop=(kp == 3),
                            perf_mode=DR,
                        )
                kto = kto_pool.tile([P, SI], fp8, tag="kto", name=f"kto{h}_{c}")
                nc.scalar.copy(kto[:], ps[:])
                nc.scalar.dma_start(out=kx[c * P:(c + 1) * P, :], in_=kto[:])
            nc.gpsimd.collective_compute(
                "AllGather", mybir.AluOpType.bypass,
                replica_groups=PAIRS,
                ins=[kx.opt()], outs=[kg.opt()],
            )
            kt_sb = kt_pool.tile([P, ET, S], fp8, tag="kt", name=f"kt{h}")
            # 16 contiguous 128KB block reads (kg[g] is [ET*P, SI] row-major,
            # so each c-block [P, SI] is one contiguous chunk)
            for g in range(2):
                for c in range(ET):
                    nc.gpsimd.dma_start(
                        out=kt_sb[:, c, g * SI:(g + 1) * SI],
                        in_=kg[g, c * P:(c + 1) * P, :],
                    )
            kt_tiles[h] = kt_sb

        qt_tiles = [None] * H

        def emit_q(h, hp):
            # ---- Q^T = Wq^T @ x^T + bq : [e, si]
            qt_sb = hp["qt"].tile([P, ET, SI], fp8, tag="qt", name=f"qt{h}")
            qt_tiles[h] = qt_sb
            for c in range(ET):
                wq_c = wqkv_pool.tile([P, ET, P], fp8, tag="wqkv", name=f"wq{h}_{c}")
                nc.sync.dma_start(out=wq_c[:], in_=A["wqb"][h, c])
                ps = psA.tile([P, 1024], f32, tag="psA")
                for nb in range(2):
                    for kp in range(4):
                        nc.tensor.matmul(
                            ps[:, nb * 512:(nb + 1) * 512],
                            lhsT=wq_c[:, 2 * kp:2 * kp + 2, :],
                            rhs=xt_sb[:, 2 * kp:2 * kp + 2, nb * 512:(nb + 1) * 512],
                            start=(kp == 0), stop=(kp == 3),
                            perf_mode=DR,
                        )
                nc.vector.tensor_scalar(
                    qt_sb[:, c, :], ps[:],
                    scalar1=bqr_sb[:, h * ET + c:h * ET + c + 1], scalar2=None,
                    op0=Alu.add,
                )

        def emit_head(h, hp, phase_b, skip_q=False):
            """One attention head. hp: dict of per-head pools.
            phase_b: callback(t) emitted after proj chain t+2 (None for h<3)."""
            if not skip_q:
                emit_q(h, hp)
            qt_sb = qt_tiles[h]
            w1_sb = w1_tiles[h]

            # ---- A^T = exp(S^T/sqrt(D) - ln4) per sj-block : [sj, si]
            kt_sb = kt_tiles[h]
            at_sb = hp["at"].tile([P, SJT, SI], fp8, tag="at")
            for j in range(SJT):
                ps = psA.tile([P, 1024], f32, tag="psA")
                for nb in range(2):
                    for kp in range(4):
                        nc.tensor.matmul(
                            ps[:, nb * 512:(nb + 1) * 512],
                            lhsT=kt_sb[:, 2 * kp:2 * kp + 2, j * P:(j + 1) * P],
                            rhs=qt_sb[:, 2 * kp:2 * kp + 2, nb * 512:(nb + 1) * 512],
                            start=(kp == 0), stop=(kp == 3),
                            perf_mode=DR,
                        )
                nc.scalar.activation(
                    out=at_sb[:, j, :], in_=ps[:], func=Act.Exp,
                    scale=SCALE, bias=nln4_sb[:],
                )

            # ---- colsum(A^T) = softmax rowsums (carrying the same 1/4)
            cs_sb = cs_pool.tile([1, SI], f32, tag="cs", name=f"cs{h}")
            for nb in range(2):
                cs_ps = psB.tile([16, 512], f32, tag="psB", name=f"cs{h}_{nb}")
                for jp in range(8):
                    nc.tensor.matmul(
                        cs_ps[:],
                        lhsT=ones8_sb[:],
                        rhs=at_sb[:, 2 * jp:2 * jp + 2, nb * 512:(nb + 1) * 512],
                        start=(jp == 0), stop=(jp == 7),
                        perf_mode=DR,
                    )
                nc.vector.tensor_copy(cs_sb[:, nb * 512:(nb + 1) * 512],
                                      cs_ps[0:1, :])
            # [1, SI] row -> [P, SIT] partition layout via a DRAM round-trip on
            # one FIFO DMA queue (write row, gather back transposed), then 1/x
            nc.sync.dma_start(out=A["csr"][h:h + 1, :], in_=cs_sb[:])
            csr_t = bass.AP(
                tensor=A["csr"].tensor, offset=A["csr"].offset + h * SI,
                ap=[[1, P], [P, SIT]],
            )
            nc.sync.dma_start(out=csT_sb[:, h, :], in_=csr_t)
            nc.vector.reciprocal(rec_sb[:, h, :], csT_sb[:, h, :])

            # ---- K-own + exchange (first three are in the prologue)
            if h == 1:
                k_exchange(3)
            if h < H - 1:
                w1n = w1_pool.tile([P, ET, D], fp8, tag="w1", name=f"w1_{h+1}")
                nc.gpsimd.dma_start(out=w1n[:], in_=A["w1"][h + 1])
                w1_tiles[h + 1] = w1n

            # ---- M = x^T @ A^T : [d, si]
            m_sb = hp["m"].tile([P, ET, SI], fp8, tag="m")
            for dc in range(ET):
                ps = psA.tile([P, 1024], f32, tag="psA")
                for nb in range(2):
                    for jp in range(8):
                        nc.tensor.matmul(
                            ps[:, nb * 512:(nb + 1) * 512],
                            lhsT=xn_sb[:, 2 * jp:2 * jp + 2, dc * P:(dc + 1) * P],
                            rhs=at_sb[:, 2 * jp:2 * jp + 2, nb * 512:(nb + 1) * 512],
                            start=(jp == 0), stop=(jp == 7),
                            perf_mode=DR,
                        )
                if h < 3:
                    nc.vector.tensor_copy(m_sb[:, dc, :], ps[:])
                else:
                    nc.scalar.copy(m_sb[:, dc, :], ps[:])

            # ---- head^T = Wv^T @ M : [e, si]
            ht_sb = hp["ht"].tile([P, ET, SI], fp8, tag="ht")
            for eb in range(ET):
                wv_eb = wqkv_pool.tile([P, ET, P], fp8, tag="wqkv", name=f"wv{h}_{eb}")
                nc.sync.dma_start(out=wv_eb[:], in_=A["wvb"][h, eb])
                ps = psA.tile([P, 1024], f32, tag="psA")
                for nb in range(2):
                    for kp in range(4):
                        nc.tensor.matmul(
                            ps[:, nb * 512:(nb + 1) * 512],
                            lhsT=wv_eb[:, 2 * kp:2 * kp + 2, :],
                            rhs=m_sb[:, 2 * kp:2 * kp + 2, nb * 512:(nb + 1) * 512],
                            start=(kp == 0), stop=(kp == 3),
                            perf_mode=DR,
                        )
                nc.scalar.copy(ht_sb[:, eb, :], ps[:])

            # ---- proj += r_h * (head_h @ W1_h)
            for t in range(SIT):
                ps = psA.tile([P, 1024], f32, tag="psA")
                for nb in range(2):
                    for ep in range(4):
                        nc.tensor.matmul(
                            ps[:, nb * 512:(nb + 1) * 512],
                            lhsT=ht_sb[:, 2 * ep:2 * ep + 2, t * P:(t + 1) * P],
                            rhs=w1_sb[:, 2 * ep:2 * ep + 2, nb * 512:(nb + 1) * 512],
                            start=(ep == 0), stop=(ep == 3),
                            perf_mode=DR,
                        )
                if h == 0:
                    nc.vector.tensor_scalar_mul(
                        proj_sb[:, t, :], ps[:], rec_sb[:, 0, t:t + 1],
                    )
                else:
                    nc.vector.scalar_tensor_tensor(
                        out=proj_sb[:, t, :], in0=ps[:],
                        scalar=rec_sb[:, h, t:t + 1],
                        in1=proj_sb[:, t, :], op0=Alu.mult, op1=Alu.add,
                    )
                if phase_b is not None and t >= 2:
                    phase_b(t - 2)
            if phase_b is not None:
                phase_b(SIT - 2)
                phase_b(SIT - 1)
                phase_b(SIT)
                phase_b(SIT + 1)

        w1_tiles = [None] * H

        # -------- prologue: K-exchange for heads 0..2, Q for head 0 --------
        head_ctx = ExitStack()
        hp = {n: head_ctx.enter_context(tc.tile_pool(name=n, bufs=1))
              for n in ("qt", "at", "m", "ht")}
        k_exchange(0)
        k_exchange(1)
        emit_q(0, hp)
        k_exchange(2)
        # xn bulk load now that the hot-path weights are queued
        for j in range(SJT):
            eng = (nc.sync, nc.scalar)[j % 2]
            eng.dma_start(out=xn_sb[:, j, :], in_=A["xn"][j * P:(j + 1) * P, :])
        w1_0 = w1_pool.tile([P, ET, D], fp8, tag="w1", name="w1_0")
        nc.gpsimd.dma_start(out=w1_0[:], in_=A["w1"][0])
        w1_tiles[0] = w1_0

        # ---------------- heads 0..2 ----------------
        for h in range(H - 1):
            emit_head(h, hp, None, skip_q=(h < 1))
        head_ctx.close()

        # w2 row-block tiles for the z-chain (allocated from the space heads
        # 0-2 just freed; DMAs overlap head 3's attention)
        w2_pool = ctx.enter_context(tc.tile_pool(name="w2", bufs=8))
        w2_tiles = []
        for kc in range(ET):
            w2_kc = w2_pool.tile([P, D], bf16, tag="w2", name=f"w2_{kc}")
            nc.gpsimd.dma_start(out=w2_kc[:], in_=A["w2"][kc * P:(kc + 1) * P, :])
            w2_tiles.append(w2_kc)

        # ---------------- head 3 + Phase B interleaved ----------------
        with ExitStack() as lctx:
            hp = {n: lctx.enter_context(tc.tile_pool(name=n + "3", bufs=1))
                  for n in ("qt", "at", "m", "ht")}
            lnp = lctx.enter_context(tc.tile_pool(name="lnp", bufs=1))
            xr_pool = lctx.enter_context(tc.tile_pool(name="xr", bufs=3))
            u_pool = lctx.enter_context(tc.tile_pool(name="up", bufs=3))
            sq_pool = lctx.enter_context(tc.tile_pool(name="sq", bufs=1))
            yt_pool = lctx.enter_context(tc.tile_pool(name="yt", bufs=1))
            st_pool = lctx.enter_context(tc.tile_pool(name="st", bufs=8))
            ot_pool = lctx.enter_context(tc.tile_pool(name="ot", bufs=1))

            if not trivial_gbe:
                gbe_sb = lnp.tile([P, 4, D], f32, tag="gbe")
                gbe_bc = bass.AP(
                    tensor=A["gbe"].tensor, offset=A["gbe"].offset,
                    ap=[[0, P], A["gbe"].ap[0], A["gbe"].ap[1]],
                )
                nc.gpsimd.dma_start(out=gbe_sb[:], in_=gbe_bc)
            y_sb = lnp.tile([P, SIT, D], bf16, tag="y")

            xr_tiles = [None] * SIT

            def fetch_xr(t):
                if t < SIT:
                    xr = xr_pool.tile([P, D], f32, tag="xr", name=f"xr{t}")
                    nc.scalar.dma_start(out=xr[:], in_=A["xres"][t * P:(t + 1) * P, :])
                    xr_tiles[t] = xr

            def ln_stats(src, rsum):
                """-> (mu, rstd) [P,1] tiles from src [P,D] + its row-sum.
                sq runs on scalar; everything else stays on DVE to minimize
                cross-engine hops on the critical path."""
                sq = sq_pool.tile([P, D], fp8, tag="sq")
                sumsq = st_pool.tile([P, 1], f32, tag="sumsq")
                nc.scalar.activation(out=sq[:], in_=src, func=Act.Square,
                                     accum_out=sumsq[:])
                mu = st_pool.tile([P, 1], f32, tag="mu")
                nc.vector.tensor_scalar_mul(mu[:], rsum, 1.0 / D)
                # (rsum*mu - sumsq) = -D*var;  std = sqrt(-1/D * that + eps)
                nv = st_pool.tile([P, 1], f32, tag="nv")
                nc.vector.scalar_tensor_tensor(
                    out=nv[:], in0=rsum, scalar=mu[:], in1=sumsq[:],
                    op0=Alu.mult, op1=Alu.subtract,
                )
                rstd = st_pool.tile([P, 1], f32, tag="rstd")
                nc.scalar.activation(out=rstd[:], in_=nv[:], func=Act.Sqrt,
                                     scale=-1.0 / D, bias=eps_sb[:])
                nc.vector.reciprocal(rstd[:], rstd[:])
                return mu, rstd

            b2_state = {}

            def phase_b(t):
                if t < SIT:
                    phase_b1(t)
                if t >= 1 and t - 1 < SIT:
                    phase_b2(t - 1)

            def phase_b1(t):
                # u1 = (x + cvec) + proj, with row-sum accumulated in the same pass
                u1 = u_pool.tile([P, D], f32, tag="u", name=f"u1_{t}")
                rs1 = st_pool.tile([P, 1], f32, tag="rs")
                nc.vector.scalar_tensor_tensor(
                    out=u1[:], in0=xr_tiles[t][:], scalar=1.0,
                    in1=proj_sb[:, t, :], op0=Alu.mult, op1=Alu.add,
                    accum_out=rs1[:],
                )
                fetch_xr(t + 3)
                mu1, rstd1 = ln_stats(u1[:], rs1[:])
                yt_t = y_sb[:, t, :]
                nc.vector.tensor_scalar(
                    yt_t, u1[:], scalar1=mu1[:], scalar2=rstd1[:],
                    op0=Alu.subtract, op1=Alu.mult,
                )
                if not trivial_gbe:
                    nc.gpsimd.tensor_mul(yt_t, yt_t, gbe_sb[:, 0, :])
                    nc.gpsimd.tensor_add(yt_t, yt_t, gbe_sb[:, 1, :])
                # transpose this tile's 8 f-blocks -> yT columns for its z-chain
                yt_tile = yt_pool.tile([P, ET, P], bf16, tag="yt")
                pb = psB.tile([P, 1024], bf16, tag="psB")
                for fb in range(ET):
                    nc.tensor.transpose(
                        pb[:, fb * P:(fb + 1) * P], yt_t[:, fb * P:(fb + 1) * P],
                        ident[:],
                    )
                pbr = pb[:].rearrange("p (f c) -> p f c", c=P)
                nc.scalar.copy(yt_tile[:, 0:4, :], pbr[:, 0:4, :])
                nc.vector.tensor_copy(yt_tile[:, 4:8, :], pbr[:, 4:8, :])
                # z-chain: u2 = y + yhat @ W2' + bu
                ps = psA.tile([P, 1024], f32, tag="psA")
                for nb in range(2):
                    for kc in range(ET):
                        nc.tensor.matmul(
                            ps[:, nb * 512:(nb + 1) * 512],
                            lhsT=yt_tile[:, kc, :],
                            rhs=w2_tiles[kc][:, nb * 512:(nb + 1) * 512],
                            start=(kc == 0), stop=False,
                        )
                    nc.tensor.matmul(
                        ps[:, nb * 512:(nb + 1) * 512],
                        lhsT=ones_sb[:, :],
                        rhs=buv_sb[:, nb * 512:(nb + 1) * 512],
                        start=False, stop=True,
                    )
                b2_state[t] = ps

            def phase_b2(t):
                ps = b2_state.pop(t)
                u2 = u_pool.tile([P, D], f32, tag="u", name=f"u2_{t}")
                rs2 = st_pool.tile([P, 1], f32, tag="rs")
                nc.vector.scalar_tensor_tensor(
                    out=u2[:], in0=y_sb[:, t, :], scalar=1.0,
                    in1=ps[:], op0=Alu.mult, op1=Alu.add,
                    accum_out=rs2[:],
                )
                mu2, rstd2 = ln_stats(u2[:], rs2[:])
                ot = ot_pool.tile([P, D], f32, tag="ot")
                nc.vector.tensor_scalar(
                    ot[:], u2[:], scalar1=mu2[:], scalar2=rstd2[:],
                    op0=Alu.subtract, op1=Alu.mult,
                )
                if not trivial_gbe:
                    nc.gpsimd.tensor_mul(ot[:], ot[:], gbe_sb[:, 2, :])
                    nc.gpsimd.tensor_add(ot[:], ot[:], gbe_sb[:, 3, :])
                nc.sync.dma_start(out=A["out"][t * P:(t + 1) * P, :], in_=ot[:])

            for t in range(3):
                fetch_xr(t)
            emit_head(H - 1, hp, phase_b)


def _build(trivial_gbe):
    import concourse.bass as bass
    import concourse.mybir as mybir
    import concourse.tile as tile
    from concourse import bacc

    f32 = mybir.dt.float32
    bf16 = mybir.dt.bfloat16
    fp8 = mybir.dt.float8e4

    nc = bacc.Bacc("TRN2", target_bir_lowering=False, debug=False, num_devices=8)
    A = {}

    def din(name, shape, dt):
        A[name] = nc.dram_tensor(name, shape, dt, kind="ExternalInput").ap()

    din("xt", [D, SI], fp8)
    din("xn", [S, D], fp8)
    din("xres", [SI, D], f32)
    din("wqb", [H, ET, P, ET, P], fp8)
    din("wkb", [H, ET, P, ET, P], fp8)
    din("wvb", [H, ET, P, ET, P], fp8)
    din("w1", [H, P, ET, D], fp8)
    din("w2", [D, D], bf16)
    din("bqr", [P, H * ET], f32)
    din("buv", [1, D], bf16)
    din("ones8", [P, 2, 16], fp8)
    A["csr"] = nc.dram_tensor("csr", [H, SI], f32, kind="Internal").ap()
    if not trivial_gbe:
        din("gbe", [4, D], f32)
    A["out"] = nc.dram_tensor("out", [SI, D], f32, kind="ExternalOutput").ap()

    with tile.TileContext(nc) as tc:
        _emit(nc, tc, A, trivial_gbe)
    nc.compile()
    return nc


def _get_nc(trivial_gbe=True):
    key = ("nc", trivial_gbe)
    if key not in _CACHE:
        _CACHE[key] = _build(trivial_gbe)
    return _CACHE[key]


def _prep_inputs(inputs):
    x = np.ascontiguousarray(inputs["embedding_matrix"], dtype=np.float32)
    Wq = np.asarray(inputs["Wq"], np.float32)
    bq = np.asarray(inputs["bq"], np.float32)
    Wv = np.asarray(inputs["Wv"], np.float32)
    bv = np.asarray(inputs["bv"], np.float32)
    Wk = np.asarray(inputs["Wk"], np.float32)
    W1 = np.asarray(inputs["W1"], np.float32)
    b1 = np.asarray(inputs["b1"], np.float32)
    W2 = np.asarray(inputs["W2"], np.float32)
    b2 = np.asarray(inputs["b2"], np.float32)
    g1 = np.asarray(inputs["g1"], np.float32)
    be1 = np.asarray(inputs["be1"], np.float32)
    g2 = np.asarray(inputs["g2"], np.float32)
    be2 = np.asarray(inputs["be2"], np.float32)

    trivial = (
        np.array_equal(g1, np.ones(D, np.float32))
        and np.array_equal(g2, np.ones(D, np.float32))
        and np.array_equal(be1, np.zeros(D, np.float32))
        and np.array_equal(be2, np.zeros(D, np.float32))
    )

    def pack_w(W):  # [H, D, D] -> [H, ET(e-blk), P(d-in), ET(kc), P(e-in)] lhsT
        return np.ascontiguousarray(
            W.reshape(H, ET, P, ET, P).transpose(0, 3, 2, 1, 4).astype(F8)
        )

    wqb = pack_w(Wq)
    wkb = pack_w(Wk)
    wvb = pack_w(Wv)
    # [H*D, D] -> [H, P(e-in), ET(e-blk), D(f)]
    w1b = np.ascontiguousarray(
        W1.reshape(H, ET, P, D).transpose(0, 2, 1, 3).astype(F8)
    )
    w2b = np.ascontiguousarray(W2.astype(BF))
    # bq rearranged so bias for (h, e-block c) is column h*ET+c: [P, H*ET]
    bqr = np.ascontiguousarray(bq.reshape(H, ET, P).transpose(2, 0, 1).reshape(P, H * ET))
    cvec = (b1 + sum(bv[h] @ W1[h * D:(h + 1) * D] for h in range(H)))
    buv = np.ascontiguousarray(b2.reshape(1, D).astype(BF))
    ones8 = np.ones((P, 2, 16), F8)

    shared = {
        "wqb": wqb, "wkb": wkb, "wvb": wvb, "w1": w1b, "w2": w2b,
        "bqr": bqr, "buv": buv, "ones8": ones8,
    }
    if not trivial:
        shared["gbe"] = np.ascontiguousarray(np.stack([g1, be1, g2, be2]))
    in_maps = []
    for core in range(8):
        b, half = core // 2, core % 2
        own = x[b, half * SI:(half + 1) * SI]
        m = dict(shared)
        m["xn"] = np.ascontiguousarray(x[b].astype(F8))   # natural order
        m["xt"] = np.ascontiguousarray(own.T.astype(F8))  # own half only
        m["xres"] = np.ascontiguousarray(own + cvec[None, :])
        in_maps.append(m)
    return trivial, in_maps


def kernel(**inputs):
    from concourse.bass_utils import run_bass_kernel_spmd

    trivial, in_maps = _prep_inputs(inputs)
    nc = _get_nc(trivial)
    res = run_bass_kernel_spmd(nc, in_maps, core_ids=list(range(8)))
    out = np.empty((4, S, D), np.float32)
    for core in range(8):
        b, half = core // 2, core % 2
        out[b, half * SI:(half + 1) * SI] = res.results[core]["out"]
    return out
